# revision 42
# baseline (speedup 1.0000x reference)
"""Trainium2 Bass kernel for nn_ExtremeOptimizationLayer (64-branch MLP + per-branch
BatchNorm + fusion gate), SPMD across 8 NeuronCores.

Sharding: expert-parallel over the 64 branches (8 per core). Per core:
  GEMM1: h_k = relu(x @ W_k + b_k) for the 8 local branches, full batch,
         computing BN batch stats on the fly (bn_stats/bn_aggr); h stored
         to per-branch DRAM tensors (pre-BN, bf16).
  BN affine (u = gamma*rsqrt(var+eps), v = beta - mean*u) is applied to the
         h tiles on the vector engine as they stream back for GEMM2 —
         no Wf1 folding, no rank-1 zc correction, no AllReduce.
  GEMM2: batch-macro outer (7x256 + 2x128 rows), both j-chunks share each
         h tile; z partials drain to bf16 and ReduceScatter per macro.
         The final macro is 128 rows so the last (latency-critical) RS is
         small; GEMM3 wave A runs under it.
  GEMM3: per-core on the 256 local batch rows; z^T via bf16 PE transposes,
         relu+bf1 on scalar, then x @ Wf2 + bf2 -> outT.
All matmuls run in bf16 (fp32 PSUM accumulation).

DMA queues: weights (W, Wf1, wf2, zs) on gpsimd, xT + h loads on sync,
h stores / zp stores / outT on scalar.
"""

import numpy as np
import ml_dtypes

import concourse.bass as bass
import concourse.mybir as mybir
import concourse.tile as tile
from concourse import bacc
from concourse.bass_utils import run_bass_kernel_spmd
from concourse.masks import make_identity

F32 = mybir.dt.float32
BF16 = mybir.dt.bfloat16
BD = ml_dtypes.bfloat16
AF = mybir.ActivationFunctionType
ALU = mybir.AluOpType

FULL_CFG = dict(
    n_cores=8, B=2048, DI=1024, DO=1024, KT=64, DF=1024, DO2=1024,
    EPS=1e-5,
)


def _bms(B):
    """Batch macros for GEMM2. Early macros are 512 rows (1KB DMA lines for
    h tiles); later ones shrink (256, then 128+128) so the RS pieces of the
    tail trigger early and spread out on the CC stream, leaving it idle when
    the final (latency-critical) piece arrives."""
    out = []
    off = 0
    while B - off >= 1536:
        out.append((off, 512))
        off += 512
    while B - off > 512:
        out.append((off, 256))
        off += 256
    if B - off == 512:
        out += [(off, 256), (off + 256, 128), (off + 384, 128)]
    else:
        out.append((off, B - off))
    return out


def _pieces(W):
    """RS row-pieces of a macro (max 256 rows per collective)."""
    out = []
    r = 0
    while r < W:
        n = min(256, W - r)
        out.append((r, n))
        r += n
    return out


def _dims(cfg):
    d = dict(cfg)
    n_cores, B, DO, DF = cfg["n_cores"], cfg["B"], cfg["DO"], cfg["DF"]
    d["KB"] = cfg["KT"] // n_cores               # branches per core
    d["TPB"] = DO // 128                         # o-tiles per branch
    d["T"] = d["KB"] * d["TPB"]                  # local ko tiles
    d["NIT"] = cfg["DI"] // 128                  # i-tiles (GEMM1 contraction)
    d["BC"] = min(512, B)                        # GEMM1 batch chunk
    d["NBC"] = B // d["BC"]
    d["HQ"] = min(4, d["TPB"])                   # ko-tiles per ht macro-DMA
    d["NTQ"] = d["T"] // d["HQ"]
    d["TPT"] = d["TPB"] // d["HQ"]               # ht tiles per branch
    d["JW"] = min(512, DF)                       # j-chunk width (per PSUM bank)
    d["NJC"] = DF // d["JW"]
    d["BMS"] = _bms(B)                           # GEMM2 batch macros
    d["BL"] = B // n_cores                       # local batch rows after RS
    d["NJT"] = DF // 128
    d["NCT"] = cfg["DO2"] // 128
    d["W1A"] = min(d["T"], 54)                   # Wf1 tiles prefetched in GEMM1
    d["W1B"] = d["T"] - d["W1A"]
    d["HT_BUFS"] = 6
    d["PRE"] = min(4, d["NTQ"])                  # bm0 ht tiles prefetched early
    return d


def build_bass(cfg):
    d = _dims(cfg)
    n_cores, B, DI, DO = cfg["n_cores"], cfg["B"], cfg["DI"], cfg["DO"]
    DF, DO2, EPS = cfg["DF"], cfg["DO2"], cfg["EPS"]
    KB, TPB, T, NIT = d["KB"], d["TPB"], d["T"], d["NIT"]
    BC, NBC = d["BC"], d["NBC"]
    HQ, NTQ, TPT = d["HQ"], d["NTQ"], d["TPT"]
    JW, NJC, BMS, BL = d["JW"], d["NJC"], d["BMS"], d["BL"]
    NJT, NCT = d["NJT"], d["NCT"]
    W1A, W1B, HT_BUFS, PRE = d["W1A"], d["W1B"], d["HT_BUFS"], d["PRE"]

    nc = bacc.Bacc("TRN2", target_bir_lowering=False, debug=False,
                   num_devices=n_cores)

    xT = nc.dram_tensor("xT", [DI, B], BF16, kind="ExternalInput").ap()
    wloc = nc.dram_tensor("wloc", [KB * DI, DO], BF16, kind="ExternalInput").ap()
    w1loc = nc.dram_tensor("w1loc", [KB * DO, DF], BF16, kind="ExternalInput").ap()
    wf2 = nc.dram_tensor("wf2", [DF, DO2], BF16, kind="ExternalInput").ap()
    b_r = nc.dram_tensor("b_r", [128, T], F32, kind="ExternalInput").ap()
    gamma_r = nc.dram_tensor("gamma_r", [128, T], F32, kind="ExternalInput").ap()
    beta_r = nc.dram_tensor("beta_r", [128, T], F32, kind="ExternalInput").ap()
    bf1_r = nc.dram_tensor("bf1_r", [128, NJT], F32, kind="ExternalInput").ap()
    bf2_r = nc.dram_tensor("bf2_r", [128, NCT], F32, kind="ExternalInput").ap()
    outT = nc.dram_tensor("outT", [DO2, BL], F32, kind="ExternalOutput").ap()

    # per-branch h (pre-BN, bf16) so GEMM2 loads only depend on one branch
    h_k = [nc.dram_tensor(f"h_{kb}", [TPB * 128, B], BF16, kind="Internal").ap()
           for kb in range(KB)]

    # fp32 z partials (CC cores reduce fp32 faster per element than bf16),
    # one RS piece per (macro, j-chunk, <=256 rows) so pieces stay small and
    # the final latency-critical one is only [128, JW]
    zp = {}
    zs = {}
    for bmi, (c0, W) in enumerate(BMS):
        for jc in range(NJC):
            for sp, (r0, nr) in enumerate(_pieces(W)):
                zp[(bmi, jc, sp)] = nc.dram_tensor(
                    f"zp{bmi}_{jc}_{sp}", [nr, JW], F32, kind="Internal").ap()
                zs[(bmi, jc, sp)] = nc.dram_tensor(
                    f"zs{bmi}_{jc}_{sp}", [nr // n_cores, JW], F32,
                    kind="Internal").ap()

    with tile.TileContext(nc) as tc:
        with tc.tile_pool(name="const", bufs=1) as cp, \
             tc.tile_pool(name="stats", bufs=1) as sp_pool, \
             tc.tile_pool(name="w1a", bufs=max(W1A, 1)) as w1a, \
             tc.tile_pool(name="ht", bufs=HT_BUFS) as htp:
            br_sb = cp.tile([128, T], F32, name="br_sb")
            gr_sb = cp.tile([128, T], F32, name="gr_sb")
            be_sb = cp.tile([128, T], F32, name="be_sb")
            bf1_sb = cp.tile([128, NJT], F32, name="bf1_sb")
            bf2_sb = cp.tile([128, NCT], F32, name="bf2_sb")
            eps_sb = cp.tile([128, 1], F32, name="eps_sb")
            ident = cp.tile([128, 128], F32, name="ident")

            # stats
            mv = sp_pool.tile([128, T, 2], F32, name="mv")
            u_t = [sp_pool.tile([128, TPB], F32, name=f"u_{kb}")
                   for kb in range(KB)]
            v_t = [sp_pool.tile([128, TPB], F32, name=f"v_{kb}")
                   for kb in range(KB)]

            w1_tiles = [None] * T
            ht_tiles = {}  # (bmi, jc, tq) -> tile

            def load_ht(bmi, jc, tq, eng):
                c0, W = BMS[bmi]
                kb = (tq * HQ) // TPB
                r0 = ((tq * HQ) % TPB) * 128
                ht = htp.tile([128, HQ, W], BF16, name=f"ht_{bmi}_{jc}_{tq}",
                              tag="ht")
                eng.dma_start(
                    ht[:],
                    h_k[kb][r0:r0 + HQ * 128, c0:c0 + W]
                    .rearrange("(q p) b -> p q b", p=128))
                ht_tiles[(bmi, jc, tq)] = ht

            def chunks_of(kb):
                return [(bc * BC, BC) for bc in range(NBC)]

            # ---------------- GEMM1: branch MLPs + BN stats ----------------
            with tc.tile_pool(name="xt", bufs=1) as xtp, \
                 tc.tile_pool(name="w", bufs=14) as wp, \
                 tc.tile_pool(name="h1", bufs=10) as hp, \
                 tc.tile_pool(name="bn", bufs=2 * TPB + 2) as bnp, \
                 tc.tile_pool(name="g1ps", bufs=6, space="PSUM") as g1ps:
                # branch-0 weights split across queues for fast start
                w_tiles = []
                for it in range(NIT):
                    wt = wp.tile([128, DO], BF16, name=f"w_0_{it}", tag="w")
                    eng = nc.sync if it == 0 else (
                        nc.gpsimd if it % 2 == 0 else nc.scalar)
                    eng.dma_start(wt[:], wloc[it * 128:(it + 1) * 128, :])
                    w_tiles.append(wt)
                nc.gpsimd.dma_start(br_sb[:], b_r[:, :])
                nc.gpsimd.dma_start(gr_sb[:], gamma_r[:, :])
                nc.gpsimd.dma_start(be_sb[:], beta_r[:, :])
                nc.gpsimd.memset(eps_sb[:], EPS)
                # xT in branch-0 consumption order, alternating sync/vector
                # queues (first thin chunk split sync/scalar)
                # xT 2/3 on sync, 1/3 on gpsimd (scalar is reserved for the
                # h-store stream; sync alone can't keep up with branch 0)
                xt_sb = xtp.tile([128, NIT, B], BF16, name="xt_sb")
                k = 0
                for ci, (off, wd) in enumerate(chunks_of(0)):
                    for it in range(NIT):
                        eng = nc.gpsimd if k % 2 == 1 else nc.sync
                        k += 1
                        eng.dma_start(
                            xt_sb[:, it, off:off + wd],
                            xT[it * 128:(it + 1) * 128, off:off + wd])

                w1_per_branch = (W1A + KB - 2) // max(KB - 1, 1)
                for kb in range(KB):
                    if kb > 0:
                        w_tiles = []
                        for it in range(NIT):
                            wt = wp.tile([128, DO], BF16, name=f"w_{kb}_{it}",
                                         tag="w")
                            nc.gpsimd.dma_start(
                                wt[:],
                                wloc[kb * DI + it * 128:kb * DI + (it + 1) * 128, :])
                            w_tiles.append(wt)
                        # prefetch bm0/jc0 h tiles of the previous branch (its
                        # h stores are all emitted by now)
                        for tq in range((kb - 1) * TPT, kb * TPT):
                            if tq < PRE:
                                load_ht(0, 0, tq, nc.sync)
                        # spread Wf1 prefetch over branches 1.. (keeps branch-0
                        # HBM bandwidth for the xT/W startup ramp)
                        for t in range((kb - 1) * w1_per_branch,
                                       min(kb * w1_per_branch, W1A)):
                            w1t = w1a.tile([128, DF], BF16, name=f"w1_{t}",
                                           tag="w1a")
                            nc.gpsimd.dma_start(
                                w1t[:], w1loc[t * 128:(t + 1) * 128, :])
                            w1_tiles[t] = w1t
                    chs = chunks_of(kb)
                    bn6s = [bnp.tile([128, len(chs), 6], F32,
                                     name=f"bn6_{kb * TPB + ot}", tag="bn6")
                            for ot in range(TPB)]
                    # branch 0 runs bc-outer so the first matmuls only need the
                    # first xT batch-chunk; later branches run ot-outer
                    if kb == 0:
                        loop = [(ot, ci) for ci in range(len(chs))
                                for ot in range(TPB)]
                    else:
                        loop = [(ot, ci) for ot in range(TPB)
                                for ci in range(len(chs))]
                    for ot, ci in loop:
                        off, wd = chs[ci]
                        t = kb * TPB + ot
                        ps = g1ps.tile([128, wd], F32, name=f"g1_{t}_{ci}",
                                       tag="g1")
                        for it in range(NIT):
                            nc.tensor.matmul(
                                ps[:],
                                w_tiles[it][:, ot * 128:(ot + 1) * 128],
                                xt_sb[:, it, off:off + wd],
                                start=(it == 0), stop=(it == NIT - 1))
                        hsb = hp.tile([128, wd], BF16, name=f"h_{t}_{ci}",
                                      tag="h1")
                        nc.scalar.activation(hsb[:], ps[:], AF.Relu,
                                             bias=br_sb[:, t:t + 1])
                        nc.vector.bn_stats(bn6s[ot][:, ci, :], hsb[:])
                        nc.scalar.dma_start(
                            h_k[kb][ot * 128:(ot + 1) * 128, off:off + wd],
                            hsb[:])
                    for ot in range(TPB):
                        t = kb * TPB + ot
                        nc.vector.bn_aggr(
                            mv[:, t, :],
                            bn6s[ot][:].rearrange("p a (x c) -> p (a x) c", c=3))
                    # per-branch BN affine: u = gamma*rsqrt(var+eps),
                    # v = beta - mean*u
                    t0 = kb * TPB
                    stdt = bnp.tile([128, TPB], F32, name=f"std_{kb}", tag="std")
                    nc.scalar.activation(stdt[:], mv[:, t0:t0 + TPB, 1:2],
                                         AF.Sqrt, bias=eps_sb[:])
                    invt = bnp.tile([128, TPB], F32, name=f"inv_{kb}", tag="inv")
                    nc.vector.reciprocal(invt[:], stdt[:])
                    nc.vector.tensor_mul(u_t[kb][:], invt[:],
                                         gr_sb[:, t0:t0 + TPB])
                    mut = bnp.tile([128, TPB], F32, name=f"mu_{kb}", tag="mu")
                    nc.vector.tensor_mul(mut[:], mv[:, t0:t0 + TPB, 0:1],
                                         u_t[kb][:])
                    nc.vector.tensor_sub(v_t[kb][:],
                                         be_sb[:, t0:t0 + TPB], mut[:])

            # ---------------- GEMM2: fusion gate partials + RS ----------------
            with tc.tile_pool(name="w1b", bufs=max(W1B, 1)) as w1b, \
                 tc.tile_pool(name="zsb", bufs=4) as zsbp, \
                 tc.tile_pool(name="zr", bufs=1) as zrp, \
                 tc.tile_pool(name="zsl", bufs=2) as zslp, \
                 tc.tile_pool(name="wf2p", bufs=1) as wf2p_pool, \
                 tc.tile_pool(name="fo", bufs=6) as fop, \
                 tc.tile_pool(name="zps", bufs=4, space="PSUM") as zps, \
                 tc.tile_pool(name="fin_ps", bufs=2, space="PSUM") as finp, \
                 tc.tile_pool(name="tp_ps", bufs=2, space="PSUM") as tpp:
                nc.gpsimd.dma_start(bf1_sb[:], bf1_r[:, :])
                nc.gpsimd.dma_start(bf2_sb[:], bf2_r[:, :])
                make_identity(nc, ident[:])
                # remaining Wf1 tiles (first used at t=W1A in bm0)
                for t in range(W1A, T):
                    w1t = w1b.tile([128, DF], BF16, name=f"w1_{t}", tag="w1b")
                    nc.gpsimd.dma_start(w1t[:], w1loc[t * 128:(t + 1) * 128, :])
                    w1_tiles[t] = w1t
                wf2_sb = wf2p_pool.tile([128, NJT, DO2], BF16, name="wf2_sb")
                for jt in range(NJT):
                    nc.gpsimd.dma_start(wf2_sb[:, jt, :],
                                        wf2[jt * 128:(jt + 1) * 128, :])

                # final-phase state
                zsl = {}        # bmi -> [W/8, DF] bf16 local z rows
                zrT = [zrp.tile([128, BL], BF16, name=f"zrT_{jt}",
                                tag=f"zrT{jt}") for jt in range(NJT)]
                transposed = set()  # bmi done
                loaded = set()

                def l0_of(bmi):
                    return BMS[bmi][0] // n_cores

                def load_zs(bmi):
                    if bmi in loaded:
                        return
                    loaded.add(bmi)
                    W = BMS[bmi][1]
                    zt = zslp.tile([W // n_cores, DF], F32, name=f"zsl_{bmi}",
                                   tag="zsl")
                    zsl[bmi] = zt
                    # on sync: the gpsimd queue head can be blocked by a
                    # pending RS trigger, which would delay this load and
                    # stall the PE at the next transpose
                    for jc in range(NJC):
                        for sp, (r0, nr) in enumerate(_pieces(W)):
                            lr = r0 // n_cores
                            nc.sync.dma_start(
                                zt[lr:lr + nr // n_cores,
                                   jc * JW:(jc + 1) * JW],
                                zs[(bmi, jc, sp)][:, :])

                def transpose_bm(bmi):
                    if bmi in transposed:
                        return
                    transposed.add(bmi)
                    plp = BMS[bmi][1] // n_cores
                    l0 = l0_of(bmi)
                    for jt in range(NJT):
                        tp = tpp.tile([128, plp], F32, name=f"tp_{bmi}_{jt}",
                                      tag="tp")
                        nc.tensor.transpose(
                            tp[:], zsl[bmi][:, jt * 128:(jt + 1) * 128],
                            ident[0:plp, 0:plp])
                        nc.scalar.activation(zrT[jt][:, l0:l0 + plp], tp[:],
                                             AF.Relu, bias=bf1_sb[:, jt:jt + 1])

                nbm = len(BMS)
                for bmi, (c0, W) in enumerate(BMS):
                    NBT = W // 128
                    # two macros back, the RS has certainly finished: pull its
                    # local rows and transpose them while GEMM2 continues
                    if bmi >= 2:
                        load_zs(bmi - 2)
                        transpose_bm(bmi - 2)
                    for jc in range(NJC):
                        z_ps = [zps.tile([128, JW], F32,
                                         name=f"z_{bmi}_{jc}_{bt}", tag="z")
                                for bt in range(NBT)]
                        for tq in range(NTQ):
                            if (bmi, jc, tq) in ht_tiles:
                                ht = ht_tiles[(bmi, jc, tq)]
                            else:
                                load_ht(bmi, jc, tq, nc.sync)
                                ht = ht_tiles[(bmi, jc, tq)]
                            for q in range(HQ):
                                t = tq * HQ + q
                                kb, ot = t // TPB, t % TPB
                                # BN affine h*u+v; alternate engines so
                                # neither falls behind the PE
                                if t % 2 == 0:
                                    nc.vector.tensor_scalar(
                                        out=ht[:, q, :], in0=ht[:, q, :],
                                        scalar1=u_t[kb][:, ot:ot + 1],
                                        scalar2=v_t[kb][:, ot:ot + 1],
                                        op0=ALU.mult, op1=ALU.add)
                                else:
                                    nc.scalar.activation(
                                        ht[:, q, :], ht[:, q, :], AF.Identity,
                                        bias=v_t[kb][:, ot:ot + 1],
                                        scale=u_t[kb][:, ot:ot + 1])
                                for bt in range(NBT):
                                    nc.tensor.matmul(
                                        z_ps[bt][:],
                                        ht[:, q, bt * 128:(bt + 1) * 128],
                                        w1_tiles[t][:, jc * JW:(jc + 1) * JW],
                                        start=(t == 0), stop=(t == T - 1),
                                        skip_group_check=True)
                        for bt in range(NBT):
                            zsb = zsbp.tile([128, JW], F32,
                                            name=f"zsb_{bmi}_{jc}_{bt}",
                                            tag="zsb")
                            nc.vector.tensor_copy(zsb[:], z_ps[bt][:])
                            sp, (r0, nr) = next(
                                (i, p) for i, p in enumerate(_pieces(W))
                                if p[0] <= bt * 128 < p[0] + p[1])
                            nc.scalar.dma_start(
                                zp[(bmi, jc, sp)][bt * 128 - r0:
                                                  bt * 128 - r0 + 128, :],
                                zsb[:])
                            if bt * 128 - r0 + 128 == nr:
                                nc.gpsimd.collective_compute(
                                    "ReduceScatter", ALU.add,
                                    replica_groups=[list(range(n_cores))],
                                    ins=[zp[(bmi, jc, sp)].opt()],
                                    outs=[zs[(bmi, jc, sp)].opt()])
                        # queue the second-to-last macro's zs load on gpsimd
                        # before the final RS triggers block that queue; its
                        # transposes go after the last matmuls (below)
                        if bmi == nbm - 1 and jc == 0 and nbm >= 2:
                            load_zs(nbm - 2)
                    if bmi == nbm - 1 and nbm >= 2:
                        transpose_bm(nbm - 2)

                lb0 = BL - BMS[-1][1] // n_cores

                def gemm3(c_lo, c_hi):
                    for ct in range(NCT):
                        ps2 = finp.tile([128, BL], F32, name=f"fo_{ct}_{c_lo}",
                                        tag="fin")
                        for jt in range(NJT):
                            nc.tensor.matmul(
                                ps2[:, c_lo:c_hi],
                                wf2_sb[:, jt, ct * 128:(ct + 1) * 128],
                                zrT[jt][:, c_lo:c_hi], start=(jt == 0),
                                stop=(jt == NJT - 1), skip_group_check=True)
                        osb = fop.tile([128, BL], F32, name=f"osb_{ct}_{c_lo}",
                                       tag="osb")
                        nc.vector.tensor_scalar_add(osb[:, c_lo:c_hi],
                                                    ps2[:, c_lo:c_hi],
                                                    bf2_sb[:, ct:ct + 1])
                        nc.scalar.dma_start(
                            outT[ct * 128:(ct + 1) * 128, c_lo:c_hi],
                            osb[:, c_lo:c_hi])

                # wave A runs while the final macro's RS is in flight
                gemm3(0, lb0)
                load_zs(nbm - 1)
                transpose_bm(nbm - 1)
                gemm3(lb0, BL)

    return nc


def prep_in_maps(cfg, x, W, b, gamma, beta, Wf1, bf1, Wf2, bf2):
    d = _dims(cfg)
    n_cores, DI, DO, DF = cfg["n_cores"], cfg["DI"], cfg["DO"], cfg["DF"]
    KB, T, TPB, NJT, NCT = d["KB"], d["T"], d["TPB"], d["NJT"], d["NCT"]

    xTb = np.ascontiguousarray(np.asarray(x, dtype=np.float32).T.astype(BD))
    wf2b = np.ascontiguousarray(np.asarray(Wf2, dtype=np.float32).astype(BD))
    bf1_rr = np.ascontiguousarray(
        np.asarray(bf1, dtype=np.float32).reshape(NJT, 128).T)
    bf2_rr = np.ascontiguousarray(
        np.asarray(bf2, dtype=np.float32).reshape(NCT, 128).T)

    def fold_cols(a_loc):  # [KB, DO] -> [128, T] with col = kb*TPB+ot
        return np.ascontiguousarray(
            np.asarray(a_loc, dtype=np.float32)
            .reshape(KB, TPB, 128).transpose(2, 0, 1).reshape(128, T))

    in_maps = []
    for c in range(n_cores):
        ks = slice(c * KB, (c + 1) * KB)
        wl = np.ascontiguousarray(
            np.asarray(W[ks], dtype=np.float32).reshape(KB * DI, DO).astype(BD))
        w1l = np.ascontiguousarray(
            np.asarray(Wf1[c * KB * DO:(c + 1) * KB * DO], dtype=np.float32)
            .astype(BD))
        in_maps.append({
            "xT": xTb, "wloc": wl, "w1loc": w1l, "wf2": wf2b,
            "b_r": fold_cols(b[ks]), "gamma_r": fold_cols(gamma[ks]),
            "beta_r": fold_cols(beta[ks]),
            "bf1_r": bf1_rr, "bf2_r": bf2_rr,
        })
    return in_maps


def assemble_output(cfg, results):
    d = _dims(cfg)
    B, DO2, n_cores = cfg["B"], cfg["DO2"], cfg["n_cores"]
    out = np.empty((B, DO2), dtype=np.float32)
    for c in range(n_cores):
        oc = results[c]["outT"].T  # [BL, DO2]
        for (c0, W) in d["BMS"]:
            for (r0, nr) in _pieces(W):
                plp = nr // n_cores
                l0 = (c0 + r0) // n_cores
                g0 = c0 + r0 + c * plp
                out[g0:g0 + plp, :] = oc[l0:l0 + plp, :]
    return out


_COMPILED = None


def _get_compiled():
    global _COMPILED
    if _COMPILED is None:
        nc = build_bass(FULL_CFG)
        nc.compile()
        _COMPILED = nc
    return _COMPILED


def kernel(**inputs):
    cfg = FULL_CFG
    nc = _get_compiled()
    in_maps = prep_in_maps(cfg, **inputs)
    res = run_bass_kernel_spmd(nc, in_maps,
                               core_ids=list(range(cfg["n_cores"])))
    return assemble_output(cfg, res.results)
